# revision 1
# baseline (speedup 1.0000x reference)
"""Local (causal) attention block on 8 TRN2 NeuronCores.

Reference computation (B=2, T=2048, C=1024, H=16, D=64):
    q,k,v = x@Wq.T, x@Wk.T, x@Wv.T          (per-head D=64)
    att   = softmax(causal_mask(q k^T / sqrt(D)))
    out   = (att v) @ Wo.T
(The reference's "window" band mask reduces exactly to the plain strict
causal mask, so this is full causal attention.)

Sharding (SPMD-uniform across the 8 cores):
  core c: batch b = c//4, head-group g = c%4 (heads 4g..4g+3),
  output-channel group g (columns 256g..256g+255).
  - QKV projections head-sharded: each core computes q^T,K^T,V for its 4
    heads, all 2048 positions (f32r matmuls on f32 inputs).
  - Attention: full causal for its 4 heads (S^T layout: kv on partitions,
    q on free axis; exp on ScalarE; rowsum via a ones-column appended to V;
    normalization via gpsimd partition-broadcast of 1/rowsum).
  - O^T (bf16) exchanged between the 4 cores of a batch with an AllGather.
  - Output projection oc-sharded: each core computes out[:, 256g:256g+256]
    for the whole batch (bf16 matmuls).
Host side only shards inputs and concatenates the per-core output slices.
"""

import sys

for _p in ("/opt/trn_rl_repo",):
    if _p not in sys.path:
        sys.path.append(_p)

import numpy as np

import concourse.bass as bass
import concourse.mybir as mybir
import concourse.tile as tile
from concourse import bacc
from concourse.bass import ts
from concourse.bass_utils import run_bass_kernel_spmd

B, T, C = 2, 2048, 1024
H, D = 16, 64
SCALE = 1.0 / np.sqrt(D)
N_CORES = 8
HPC = H // 4          # heads per core = 4
COC = C // 4          # channels per core = 256
F32 = mybir.dt.float32
BF16 = mybir.dt.bfloat16
F32R = mybir.dt.float32r
NEG = -1.0e5          # additive mask value (pre-scale)


def r(ap):
    """view an f32 AP as f32r for full-rate fp32 matmul"""
    return ap.bitcast(F32R)


def attention_qchunk(nc, psum, work, qT_sb, kT_sb, v_sb, masks, otall_c, qc):
    """Causal attention for one 512-wide q-chunk, all 4 local heads.

    S^T layout (kv on partitions, q free). Head pairs are row-packed on
    the PE (rows 0-63 / 64-127 via tile_position). Softmax normalization
    is batched per chunk: the V ones-column puts each head's rowsum in
    psum row D; rowsums are collected into a [4, 512] tile, reciprocated
    in one DVE op, partition-broadcast, and multiplied into O^T.
    """
    nkv = 4 * (qc + 1)
    # rowsums parked on partitions {0,32,64,96} (the only legal AP start
    # partitions); memset so the batched reciprocal sees no garbage
    rsums = work.tile([128, 512], F32, tag="rsums")
    nc.vector.memset(rsums[:], 1.0)
    ot_ps = {}
    for pair in range(HPC // 2):
        h0, h1 = 2 * pair, 2 * pair + 1
        for i in (0, 1):
            ot_ps[2 * pair + i] = psum.tile(
                [D + 1, 512], F32, tag=f"o{i}", name=f"o{i}"
            )
        for k in range(nkv):
            s_ps = [
                psum.tile([128, 512], F32, tag=f"s{i}", name=f"s{i}")
                for i in (0, 1)
            ]
            for i, h in ((0, h0), (1, h1)):
                bp = 64 * (h % 2)
                nc.tensor.matmul(
                    s_ps[i][:],
                    kT_sb[bp : bp + 64, h // 2, ts(k, 128)],
                    qT_sb[bp : bp + 64, h // 2, ts(qc, 512)],
                    start=True,
                    stop=True,
                    tile_position=(bp, 0),
                )
            m = k - 4 * qc
            for i, h in ((0, h0), (1, h1)):
                pt = work.tile([128, 512], BF16, tag="pt")
                nc.scalar.activation(
                    pt[:],
                    s_ps[i][:],
                    mybir.ActivationFunctionType.Exp,
                    scale=float(SCALE),
                )
                if m >= 0:  # diagonal tile: zero the disallowed region
                    nc.vector.tensor_mul(pt[:], pt[:], masks[:, m, :])
                nc.tensor.matmul(
                    ot_ps[h][:],
                    v_sb[:, k, h, :],
                    pt[:],
                    start=(k == 0),
                    stop=(k == nkv - 1),
                )
        for i, h in ((0, h0), (1, h1)):
            nc.vector.tensor_copy(
                rsums[32 * h : 32 * h + 1, :], ot_ps[h][D : D + 1, :]
            )
    nc.vector.reciprocal(rsums[:], rsums[:])
    for h in range(HPC):
        # stage this head's reciprocals at partition 0: partition_broadcast
        # reads partition 0 of its source on hardware
        stg = work.tile([1, 512], F32, tag="stg")
        nc.vector.tensor_copy(stg[:], rsums[32 * h : 32 * h + 1, :])
        bcast = work.tile([64, 512], F32, tag="bcast")
        nc.gpsimd.partition_broadcast(bcast[:], stg[:])
        nc.vector.tensor_mul(
            otall_c[64 * (h % 2) : 64 * (h % 2) + 64, h // 2, :],
            ot_ps[h][0:D, :],
            bcast[:],
        )


def build_nc():
    nc = bacc.Bacc(
        "TRN2",
        target_bir_lowering=False,
        debug=False,
        num_devices=N_CORES,
    )
    xT_d = nc.dram_tensor("xT", [C, T], F32, kind="ExternalInput").ap()
    wqT_d = nc.dram_tensor("wqT", [C, COC], F32, kind="ExternalInput").ap()
    wkT_d = nc.dram_tensor("wkT", [C, COC], F32, kind="ExternalInput").ap()
    wvT_d = nc.dram_tensor("wvT", [C, COC], F32, kind="ExternalInput").ap()
    woT_d = nc.dram_tensor("woT", [C, COC], F32, kind="ExternalInput").ap()
    out_d = nc.dram_tensor("out", [T, COC], F32, kind="ExternalOutput").ap()

    NQC = T // 512     # 4 q-chunks of 512
    NKT = T // 128     # 16 kv tiles of 128
    NCT = C // 128     # 8 contraction tiles

    with tile.TileContext(nc) as tc:
        with (
            tc.tile_pool(name="main", bufs=1) as main,
            tc.tile_pool(name="work", bufs=4) as work,
            tc.tile_pool(name="dram", bufs=2, space="DRAM") as dram,
        ):
            # ---- long-lived SBUF tensors ----
            qT_sb = main.tile([128, 2, T], BF16)         # [co 256, t]
            kT_sb = main.tile([128, 2, T], BF16)
            v_sb = main.tile([128, NKT, HPC, D + 1], BF16)  # V + ones col
            # own normalized O^T / gathered O^T, one tile per q-chunk so the
            # exchange + output projection pipeline per chunk
            otall_c = [
                main.tile([128, 2, 512], BF16, name=f"otall{j}") for j in range(NQC)
            ]
            otfull_c = [
                main.tile([128, NCT, 512], BF16, name=f"otfull{j}")
                for j in range(NQC)
            ]
            woT_bf = main.tile([128, NCT, COC], BF16)
            masks = main.tile([128, 4, 512], BF16)       # 1/0 multiplicative

            # ---- phase 1: projections (f32r), x^T streamed in t-chunks ----
            xT_r = xT_d.rearrange("(a p) t -> p a t", p=128)
            with (
                tc.tile_pool(name="p1w", bufs=1) as p1w,
                tc.tile_pool(name="p1ws", bufs=2) as p1ws,
                tc.tile_pool(name="p1x", bufs=2) as p1x,
                tc.tile_pool(name="p1psum", bufs=2, space="PSUM") as psum_p1,
            ):
                wq_sb = p1w.tile([128, NCT, COC], BF16)
                wk_sb = p1w.tile([128, NCT, COC], BF16)
                wv_sb = p1w.tile([128, NCT, COC], BF16)
                for w_sb, w_d in ((wq_sb, wqT_d), (wk_sb, wkT_d), (wv_sb, wvT_d)):
                    wst = p1ws.tile([128, NCT, COC], F32, tag="wst")
                    nc.sync.dma_start(
                        out=wst[:], in_=w_d.rearrange("(a p) t -> p a t", p=128)
                    )
                    nc.vector.tensor_copy(w_sb[:], wst[:])

                for tj in range(NQC):
                    xch = p1x.tile([128, NCT, 512], F32, tag="xch")
                    nc.sync.dma_start(out=xch[:], in_=xT_r[:, :, ts(tj, 512)])
                    xbf = p1x.tile([128, NCT, 512], BF16, tag="xbf")
                    nc.vector.tensor_copy(xbf[:], xch[:])

                    # q^T and K^T: [co, t] = sum_c W[c, co]^T x^T[c, t]
                    for w_sb, dst in ((wq_sb, qT_sb), (wk_sb, kT_sb)):
                        for co in range(2):
                            ps = psum_p1.tile([128, 512], F32, tag="psA")
                            for ci in range(NCT):
                                nc.tensor.matmul(
                                    ps[:],
                                    w_sb[:, ci, ts(co, 128)],
                                    xbf[:, ci, :],
                                    start=(ci == 0),
                                    stop=(ci == NCT - 1),
                                )
                            nc.vector.tensor_copy(dst[:, co, ts(tj, 512)], ps[:])

                    # V: [t, co] = sum_c x^T[c, t]^T W_v^T[c, co]; aug layout
                    for tl in range(4):
                        tt = 4 * tj + tl
                        ps = psum_p1.tile([128, COC], F32, tag="psB")
                        for ci in range(NCT):
                            nc.tensor.matmul(
                                ps[:],
                                xbf[:, ci, ts(tl, 128)],
                                wv_sb[:, ci, :],
                                start=(ci == 0),
                                stop=(ci == NCT - 1),
                            )
                        nc.vector.tensor_copy(
                            v_sb[:, tt, :, 0:D],
                            ps[:].rearrange("p (h d) -> p h d", h=HPC),
                        )
                nc.vector.memset(v_sb[:, :, :, D], 1.0)

            # weights for the output projection (needed from chunk 0)
            wo_f32 = main.tile([128, NCT, COC], F32)
            nc.sync.dma_start(
                out=wo_f32[:], in_=woT_d.rearrange("(a p) t -> p a t", p=128)
            )
            for ci in range(NCT):
                nc.vector.tensor_copy(woT_bf[:, ci, :], wo_f32[:, ci, :])

            # ---- phase 2-4 pipeline per q-chunk:
            #      attention -> AllGather(O^T chunk) -> out projection ----
            mk32 = main.tile([128, 4, 512], F32)
            for m in range(4):
                nc.gpsimd.memset(mk32[:, m, :], 1.0)
                nc.gpsimd.affine_select(
                    out=mk32[:, m, :],
                    in_=mk32[:, m, :],
                    pattern=[[1, 512]],
                    compare_op=mybir.AluOpType.is_ge,
                    fill=0.0,
                    base=-128 * m,
                    channel_multiplier=-1,
                )
                nc.vector.tensor_copy(masks[:, m, :], mk32[:, m, :])

            with tc.tile_pool(name="psum2", bufs=2, space="PSUM") as psum:

                def out_proj_chunk(qc):
                    # oc-sharded output projection for one 512-row q-chunk
                    for ql in range(4):
                        ps = psum.tile(
                            [128, COC], F32, tag=f"s{ql % 2}", name="po"
                        )
                        for ci in range(NCT):
                            nc.tensor.matmul(
                                ps[:],
                                otfull_c[qc][:, ci, ts(ql, 128)],
                                woT_bf[:, ci, :],
                                start=(ci == 0),
                                stop=(ci == NCT - 1),
                            )
                        ot = work.tile([128, COC], F32, tag="outst")
                        nc.vector.tensor_copy(ot[:], ps[:])
                        nc.sync.dma_start(
                            out=out_d[ts(4 * qc + ql, 128), :], in_=ot[:]
                        )

                for qc in range(NQC):
                    attention_qchunk(
                        nc, psum, work, qT_sb, kT_sb, v_sb, masks, otall_c[qc], qc
                    )

                    # exchange this chunk's O^T across the batch's 4 cores
                    bounce_in = dram.tile(
                        [COC, 512], BF16, tag="bin", name=f"bin{qc}"
                    )
                    bounce_out = dram.tile(
                        [C, 512], BF16, tag="bout", name=f"bout{qc}"
                    )
                    for i in range(2):
                        nc.sync.dma_start(
                            out=bounce_in[ts(i, 128), :], in_=otall_c[qc][:, i, :]
                        )
                    nc.gpsimd.collective_compute(
                        "AllGather",
                        mybir.AluOpType.bypass,
                        replica_groups=[[0, 1, 2, 3], [4, 5, 6, 7]],
                        ins=[bounce_in.opt()],
                        outs=[bounce_out.opt()],
                    )
                    nc.sync.dma_start(
                        out=otfull_c[qc][:],
                        in_=bounce_out[:].rearrange("(a p) t -> p a t", p=128),
                    )
                    # emit the PREVIOUS chunk's projection here so its psum
                    # slot reuse never makes the next chunk's attention wait
                    # on this chunk's AllGather
                    if qc > 0:
                        out_proj_chunk(qc - 1)
                out_proj_chunk(NQC - 1)

    nc.compile()
    return nc


_NC_CACHE = None


def _get_nc():
    global _NC_CACHE
    if _NC_CACHE is None:
        _NC_CACHE = build_nc()
    return _NC_CACHE


def make_in_maps(x, Wq, Wk, Wv, Wo):
    x = np.asarray(x, dtype=np.float32)
    in_maps = []
    for c in range(N_CORES):
        b, g = c // 4, c % 4
        sl = slice(COC * g, COC * g + COC)
        in_maps.append(
            {
                "xT": np.ascontiguousarray(x[b].T),
                "wqT": np.ascontiguousarray(np.asarray(Wq)[sl, :].T),
                "wkT": np.ascontiguousarray(np.asarray(Wk)[sl, :].T),
                "wvT": np.ascontiguousarray(np.asarray(Wv)[sl, :].T),
                "woT": np.ascontiguousarray(np.asarray(Wo)[sl, :].T),
            }
        )
    return in_maps


def assemble(results):
    out = np.empty((B, T, C), dtype=np.float32)
    for c in range(N_CORES):
        b, g = c // 4, c % 4
        out[b, :, COC * g : COC * g + COC] = results[c]["out"]
    return out


def kernel(x, Wq, Wk, Wv, Wo):
    nc = _get_nc()
    in_maps = make_in_maps(x, Wq, Wk, Wv, Wo)
    res = run_bass_kernel_spmd(nc, in_maps, list(range(N_CORES)))
    return assemble(res.results)


if __name__ == "__main__":
    rng = np.random.default_rng(0)
    x = rng.standard_normal((B, T, C), dtype=np.float32)
    s = 1.0 / np.sqrt(C)
    ws = [
        rng.uniform(-s, s, size=(C, C)).astype(np.float32) for _ in range(4)
    ]
    out = kernel(x, *ws)
    print("kernel ran; out", out.shape, out.dtype)

